# revision 4
# baseline (speedup 1.0000x reference)
"""ConvLSTM attention pooling kernel for 8 Trainium2 NeuronCores.

Reference computation (per sample b):
    frames = x[b].reshape(chi, D)            # D = C*H*W = 65536, chi = 20
    scores = frames @ frames[-1] / chi       # [chi]
    alpha  = softmax(scores)                 # [chi]
    y      = x[b].reshape(D, chi) @ alpha    # [D]  (interleaved reshape)

Sharding: pure data-parallel over batch B=64 -> 8 samples per core.

Architecture (v5, HW-measured on the axon-tunneled TRN2 cores):
  - one contiguous fp32 load of x[b] as nt[128, 10240] (DMA roofline
    ~124 us/core for the 42 MB shard)
  - 80 fp32 PE transposes/sample -> PSUM (fp32 transposes are fast on HW;
    bf16 transposes and fp32 matmuls are NOT - fp32 stage-2 matmuls
    measured 263 us/core standalone vs 9 us for bf16)
  - dual copies from each PSUM tile:
      * flat-f32 "want" layout (strided dst, fp32 full rate) for stage 1
      * t-major block-contiguous bf16 "wbs" layout for stage 2 stationaries
        (16-bit writes are ~4x slower when the dst is strided - the wbs
        layout keeps them in contiguous runs AND makes every stage-2
        stationary a contiguous 128-col slice)
      engine split: ACT most copies, DVE a share of flat, Pool (gpsimd)
      recasts a share of the bf16 groups SBUF->SBUF
  - stage 1: 19 DVE STT dot products + 1 ACT square (frame 19), column
    reduce on PE, softmax, alpha-scatter tiles (rhs, bf16)
  - stage 2: 80 bf16 matmuls (stat = wbs slice [128,128] contiguous,
    mov = rhs [128,32]), PSUM -> SBUF -> store
  HW repeat-R marginal: ~250 us/core (prior fp32 version: ~445 us/core).
"""

import numpy as np

B = 64
CHI = 20
D = 64 * 32 * 32  # 65536
N_CORES = 8
S = B // N_CORES  # samples per core
P = 128
F = D // P  # 512

NW = 5
NG = 32
NB = 80
NCHUNK = 16
HB = NB // 4  # 128-col blocks per load chunk

_CACHE = {}


def _build_nc_y(repeat=1, n_pool_bf=8, n_dve_f32=1, nt_bufs=4, pools=None,
                big_chunks=False, rhs_one=True):
    import concourse.bacc as bacc
    import concourse.tile as tile
    from concourse import mybir

    f32 = mybir.dt.float32
    bf16 = mybir.dt.bfloat16
    nc = bacc.Bacc("TRN2", target_bir_lowering=False, debug=False)
    x_d = nc.dram_tensor("x", [S, CHI * D], f32, kind="ExternalInput").ap()
    ident_d = nc.dram_tensor("ident", [P, P], f32, kind="ExternalInput").ap()
    ind1_d = nc.dram_tensor("ind1", [5, CHI, P], f32, kind="ExternalInput").ap()
    ind2_d = nc.dram_tensor("ind2", [5, P, 32], f32, kind="ExternalInput").ap()
    y_d = nc.dram_tensor("y", [S, D], f32, kind="ExternalOutput").ap()

    pp = dict(nt=nt_bufs, wantf=2, wblk=2, scratch=2, osb=2,
              pst=5, pss=1, pso=2)
    if pools:
        pp.update(pools)

    with tile.TileContext(nc) as tc:
        with (
            tc.tile_pool(name="nt", bufs=pp["nt"]) as nt_pool,
            tc.tile_pool(name="wantf", bufs=pp["wantf"]) as wantf_pool,
            tc.tile_pool(name="wblk", bufs=pp["wblk"]) as wblk_pool,
            tc.tile_pool(name="scratch", bufs=pp["scratch"]) as s_pool,
            tc.tile_pool(name="small", bufs=6) as sm_pool,
            tc.tile_pool(name="rhs", bufs=3) as rhs_pool,
            tc.tile_pool(name="osb", bufs=pp["osb"]) as osb_pool,
            tc.tile_pool(name="singles", bufs=1) as ones_pool,
            tc.tile_pool(name="pst", bufs=pp["pst"], space="PSUM") as pst_pool,
            tc.tile_pool(name="pss", bufs=pp["pss"], space="PSUM") as pss_pool,
            tc.tile_pool(name="pso", bufs=pp["pso"], space="PSUM") as pso_pool,
        ):
            inv_chi_col = ones_pool.tile([P, 1], f32)
            nc.vector.memset(inv_chi_col, 1.0 / CHI)
            ident = ones_pool.tile([P, P], f32)
            nc.sync.dma_start(out=ident, in_=ident_d)
            ind1 = ones_pool.tile([CHI, 5, P], f32)
            nc.sync.dma_start(out=ind1, in_=ind1_d.rearrange("s c p -> c s p"))
            ind2 = ones_pool.tile([P, 5, 32], f32)
            nc.sync.dma_start(out=ind2, in_=ind2_d.rearrange("s p g -> p s g"))

            wantfs = [None] * S
            wblks = [None] * S
            rhss = [None] * S
            partials_arr = [None] * S
            scr = [None] * S

            def emit_s1_frame(b, c):
                """partials[:, c] = sum_f wf[:, c, f] * wf[:, 19, f]."""
                wf = wantfs[b].rearrange("p (c e) -> p c e", c=CHI)
                partials = partials_arr[b]
                if c == CHI - 1:
                    nc.scalar.activation(
                        out=scr[b][4],
                        in_=wf[:, CHI - 1, :],
                        func=mybir.ActivationFunctionType.Square,
                        accum_out=partials[:, CHI - 1 : CHI],
                    )
                    return
                nc.vector.scalar_tensor_tensor(
                    out=scr[b][c % 4],
                    in0=wf[:, c, :],
                    scalar=1.0,
                    in1=wf[:, CHI - 1, :],
                    op0=mybir.AluOpType.mult,
                    op1=mybir.AluOpType.mult,
                    accum_out=partials[:, c : c + 1],
                )

            def emit_stage1_dots(b):
                emit_s1_frame(b, CHI - 1)
                for c in range(CHI - 1):
                    emit_s1_frame(b, c)

            def emit_load_transpose(b):
                u = x_d[b].rearrange("(p q) -> p q", p=P)
                wantf = wantf_pool.tile([P, CHI * F], f32)
                wblk = wblk_pool.tile([P, NB * P], bf16)
                wantfs[b] = wantf
                wblks[b] = wblk
                partials_arr[b] = sm_pool.tile([P, CHI], f32, name="partials")
                scr[b] = [s_pool.tile([P, F], f32, name=f"scr{i}") for i in range(5)]
                wantf_v = wantf.rearrange("p (pp j) -> p j pp", j=NB)
                wblk_v = wblk.rearrange(
                    "p (q s t h dd) -> p q s t h dd", q=16, s=5, t=8, h=4
                )
                halves = [None, None]
                for h in range(4):
                    if big_chunks:
                        if h % 2 == 0:
                            nth = nt_pool.tile([P, 2 * HB * P], f32, name="nth")
                            nc.sync.dma_start(
                                out=nth,
                                in_=u[:, h * HB * P : (h + 2) * HB * P],
                            )
                            halves[h // 2] = nth
                        nt = halves[h // 2][:, (h % 2) * HB * P : (h % 2 + 1) * HB * P]
                    else:
                        nt = nt_pool.tile([P, HB * P], f32)
                        nc.sync.dma_start(
                            out=nt, in_=u[:, h * HB * P : (h + 1) * HB * P]
                        )
                    for sres in range(5):
                        # psum tile sres holds blocks j = 5*(4h+dd) + sres
                        ps = pst_pool.tile([P, 4, P], f32)
                        for dd in range(4):
                            jl = 5 * dd + sres  # local block within chunk h
                            nc.tensor.transpose(
                                ps[:, dd, :], nt[:, jl * P : (jl + 1) * P], ident
                            )
                        jga = h * 5 + sres
                        # flat f32 copy (for stage 1): dst j-blocks stride 5
                        j0 = 20 * h + sres
                        dst = wantf_v[:, j0 : j0 + 16 : 5, :]
                        if jga % 5 < n_dve_f32:
                            nc.vector.tensor_copy(out=dst, in_=ps)
                        else:
                            nc.scalar.copy(out=dst, in_=ps)
                        # bf16 copy for stage 2 (t-major block layout):
                        # wbs[p, 640q+128s+16t+4h+dd] = ps[p, dd, 8q+t]
                        bdst = wblk_v[:, :, sres, :, h, :]
                        if jga % 5 < -(-n_pool_bf // 4):
                            # Pool recast: SBUF->SBUF from the flat copy
                            src_b = dst.rearrange("p dd (q t) -> p q t dd", t=8)
                            nc.gpsimd.tensor_copy(out=bdst, in_=src_b)
                        else:
                            src_b = ps.rearrange("p dd (q t) -> p q t dd", t=8)
                            nc.scalar.copy(out=bdst, in_=src_b)

            def emit_stage1_rest(b):
                partials = partials_arr[b]
                soft = pss_pool.tile([P, 48], f32)
                s_psum = soft[0:1, 0:CHI]
                a_psum = soft[0:CHI, 24:25]
                a_pat = soft[:, 32:40]
                nc.tensor.matmul(s_psum, inv_chi_col, partials, start=True, stop=True)
                scores = sm_pool.tile([1, CHI], f32)
                nc.vector.tensor_copy(out=scores, in_=s_psum)

                neg_mx = sm_pool.tile([1, 1], f32)
                nc.vector.tensor_reduce(
                    out=neg_mx,
                    in_=scores,
                    axis=mybir.AxisListType.X,
                    op=mybir.AluOpType.max,
                    negate=True,
                )
                exps = sm_pool.tile([1, CHI], f32)
                sumexp = sm_pool.tile([1, 1], f32)
                nc.scalar.activation(
                    out=exps,
                    in_=scores,
                    func=mybir.ActivationFunctionType.Exp,
                    bias=neg_mx[:, 0:1],
                    scale=1.0,
                    accum_out=sumexp,
                )
                rsum = sm_pool.tile([1, 1], f32)
                nc.vector.reciprocal(rsum, sumexp)
                alpha = sm_pool.tile([1, CHI], f32)
                nc.vector.tensor_scalar_mul(alpha, exps, rsum)

                nc.tensor.transpose(a_psum, alpha, ident[:1, :1])
                a_one = sm_pool.tile([CHI, 1], f32)
                nc.scalar.copy(out=a_one, in_=a_psum)

                # a_pat[:, s] = ind1_s.T @ alpha_col = alpha[(128s+p) % 20]
                for s in range(NW):
                    nc.tensor.matmul(
                        a_pat[:, s : s + 1], ind1[:, s, :], a_one,
                        start=True, stop=True,
                    )
                # rhs_s[p, g] = ind2_s[p, g] * a_pat[p, s]  (bf16 for stage 2)
                rhs = rhs_pool.tile([P, NW, NG], bf16)
                rhss[b] = rhs
                if rhs_one:
                    a_bc = a_pat[:, 0:5].rearrange("p (s o) -> p s o", o=1)
                    nc.vector.tensor_mul(
                        rhs, ind2, a_bc.broadcast_to([P, NW, NG])
                    )
                else:
                    for s in range(NW):
                        nc.vector.tensor_scalar_mul(
                            rhs[:, s, :], ind2[:, s, :], a_pat[:, s : s + 1]
                        )

            def emit_stage2(b):
                wblk = wblks[b]
                rhs = rhss[b]
                ob = pso_pool.tile([P, NCHUNK, NG], f32)
                for W in range(NCHUNK):
                    for s in range(NW):
                        # stationary: contiguous 128 cols; free pos i=16t+jd
                        # holds want col 5*(128W + i) + s
                        stat = wblk[:, 640 * W + 128 * s : 640 * W + 128 * s + 128]
                        nc.tensor.matmul(
                            ob[:, W, :],
                            stat,
                            rhs[:, s, :],
                            start=(s == 0),
                            stop=(s == NW - 1),
                        )
                out_sb = osb_pool.tile([P, NCHUNK, NG], f32)
                nc.scalar.copy(out=out_sb, in_=ob)
                # psum partition i holds y[4096W + 32i + g]
                nc.sync.dma_start(
                    out=y_d[b].rearrange("(W m g) -> m W g", W=NCHUNK, g=NG),
                    in_=out_sb,
                )

            def emit_all():
                for b in range(S + 1):
                    if 0 <= b - 1 < S:
                        emit_stage1_dots(b - 1)
                    if b < S:
                        emit_load_transpose(b)
                    if 0 <= b - 1 < S:
                        emit_stage1_rest(b - 1)
                        emit_stage2(b - 1)

            if repeat == 1:
                emit_all()
            elif repeat < 0:  # dynamic loop (barrier per iteration)
                with tc.For_i(0, -repeat, 1):
                    emit_all()
            else:  # fully unrolled
                for _rep in range(repeat):
                    emit_all()

    nc.compile()
    return nc


def _host_inputs(xs):
    """Global (all-core concatenated) input arrays keyed by dram tensor name."""
    feed = {"x": xs}
    feed["ident"] = np.tile(np.eye(P, dtype=np.float32), (N_CORES, 1))
    s_idx = np.arange(5)[:, None]
    p_idx = np.arange(P)[None, :]
    cmap = (128 * s_idx + p_idx) % CHI  # [5, P]
    gmap = (128 * s_idx + p_idx) // CHI  # [5, P]
    ind1 = (np.arange(CHI)[None, :, None] == cmap[:, None, :]).astype(np.float32)
    ind2 = (np.arange(32)[None, None, :] == gmap[:, :, None]).astype(np.float32)
    feed["ind1"] = np.tile(ind1, (N_CORES, 1, 1))
    feed["ind2"] = np.tile(ind2, (N_CORES, 1, 1))
    return feed


def _get_nc():
    if "nc" not in _CACHE:
        _CACHE["nc"] = _build_nc_y()
    return _CACHE["nc"]


def _get_runner():
    if "runner" not in _CACHE:
        run, sharded, mesh, body = _make_runner(_get_nc())
        _CACHE["sharded"] = sharded
        _CACHE["mesh"] = mesh
        _CACHE["body"] = body
        _CACHE["runner"] = run
    return _CACHE["runner"]


def _make_runner(nc):
    """Compile once and return f(x_global[64, CHI*D]) -> y_global[64, D]."""
    import jax
    from jax.sharding import Mesh, PartitionSpec
    from jax.experimental.shard_map import shard_map
    from concourse import bass2jax, mybir

    bass2jax.install_neuronx_cc_hook()

    partition_name = (
        nc.partition_id_tensor.name if nc.partition_id_tensor else None
    )
    in_names = []
    out_names = []
    out_avals = []
    zero_outs = []
    for alloc in nc.m.functions[0].allocations:
        if not isinstance(alloc, mybir.MemoryLocationSet):
            continue
        name = alloc.memorylocations[0].name
        if alloc.kind == "ExternalInput":
            if name != partition_name:
                in_names.append(name)
        elif alloc.kind == "ExternalOutput":
            shape = tuple(alloc.tensor_shape)
            dtype = mybir.dt.np(alloc.dtype)
            out_avals.append(jax.core.ShapedArray(shape, dtype))
            out_names.append(name)
            zero_outs.append(np.zeros(shape, dtype))
    n_params = len(in_names)
    n_outs = len(out_avals)
    in_names.extend(out_names)
    donate = tuple(range(n_params, n_params + n_outs))

    def _body(*args):
        operands = list(args)
        if partition_name is not None:
            operands.append(bass2jax.partition_id_tensor())
            in_full = tuple(in_names) + (partition_name,)
        else:
            in_full = tuple(in_names)
        outs = bass2jax._bass_exec_p.bind(
            *operands,
            out_avals=tuple(out_avals),
            in_names=in_full,
            out_names=tuple(out_names),
            lowering_input_output_aliases=(),
            sim_require_finite=True,
            sim_require_nnan=True,
            nc=nc,
        )
        return tuple(outs)

    devices = jax.devices()[:N_CORES]
    mesh = Mesh(np.asarray(devices), ("core",))
    in_specs = (PartitionSpec("core"),) * (n_params + n_outs)
    out_specs = (PartitionSpec("core"),) * len(out_names)
    sharded = jax.jit(
        shard_map(
            _body, mesh=mesh, in_specs=in_specs, out_specs=out_specs, check_rep=False
        ),
        donate_argnums=donate,
        keep_unused=True,
    )

    param_names = in_names[:n_params]
    _CACHE["param_names"] = param_names

    def run(xs):
        feed = _host_inputs(xs)
        args = [feed[n] for n in param_names]
        concat_zeros = [
            np.zeros((N_CORES * z.shape[0], *z.shape[1:]), z.dtype) for z in zero_outs
        ]
        return sharded(*args, *concat_zeros)[0]

    return run, sharded, mesh, _body


def kernel(**inputs):
    x = np.ascontiguousarray(np.asarray(inputs["x"], dtype=np.float32))
    assert x.shape == (B, CHI, 64, 32, 32), x.shape
    xs = x.reshape(B, CHI * D)
    run = _get_runner()
    last_err = None
    for _attempt in range(3):
        try:
            out = np.asarray(run(xs))
            break
        except Exception as e:  # transient NRT device errors: retry
            last_err = e
    else:
        raise last_err
    return out.reshape(B, 64, 32, 32)


# revision 6
# speedup vs baseline: 1.0166x; 1.0166x over previous
"""ConvLSTM attention pooling kernel for 8 Trainium2 NeuronCores.

Reference computation (per sample b):
    frames = x[b].reshape(chi, D)            # D = C*H*W = 65536, chi = 20
    scores = frames @ frames[-1] / chi       # [chi]
    alpha  = softmax(scores)                 # [chi]
    y      = x[b].reshape(D, chi) @ alpha    # [D]  (interleaved reshape)

Sharding: pure data-parallel over batch B=64 -> 8 samples per core.

Architecture (v5, HW-measured on the axon-tunneled TRN2 cores):
  - one contiguous fp32 load of x[b] as nt[128, 10240] (DMA roofline
    ~124 us/core for the 42 MB shard)
  - 80 fp32 PE transposes/sample -> PSUM (fp32 transposes are fast on HW;
    bf16 transposes and fp32 matmuls are NOT - fp32 stage-2 matmuls
    measured 263 us/core standalone vs 9 us for bf16)
  - dual copies from each PSUM tile:
      * flat-f32 "want" layout (strided dst, fp32 full rate) for stage 1
      * t-major block-contiguous bf16 "wbs" layout for stage 2 stationaries
        (16-bit writes are ~4x slower when the dst is strided - the wbs
        layout keeps them in contiguous runs AND makes every stage-2
        stationary a contiguous 128-col slice)
      engine split: ACT most copies, DVE a share of flat, Pool (gpsimd)
      recasts a share of the bf16 groups SBUF->SBUF
  - stage 1: 19 DVE STT dot products + 1 ACT square (frame 19), column
    reduce on PE, softmax, alpha-scatter tiles (rhs, bf16)
  - stage 2: 80 bf16 matmuls (stat = wbs slice [128,128] contiguous,
    mov = rhs [128,32]), PSUM -> SBUF -> store
  HW repeat-R marginal: ~250 us/core (prior fp32 version: ~445 us/core).
"""

import numpy as np

B = 64
CHI = 20
D = 64 * 32 * 32  # 65536
N_CORES = 8
S = B // N_CORES  # samples per core
P = 128
F = D // P  # 512

NW = 5
NG = 32
NB = 80
NCHUNK = 16
HB = NB // 4  # 128-col blocks per load chunk

_CACHE = {}


def _build_nc_y(repeat=1, n_pool_bf=8, n_dve_f32=2, nt_bufs=4, pools=None,
                big_chunks=False, rhs_one=True, sq_dve=False):
    import concourse.bacc as bacc
    import concourse.tile as tile
    from concourse import mybir

    f32 = mybir.dt.float32
    bf16 = mybir.dt.bfloat16
    nc = bacc.Bacc("TRN2", target_bir_lowering=False, debug=False)
    x_d = nc.dram_tensor("x", [S, CHI * D], f32, kind="ExternalInput").ap()
    ident_d = nc.dram_tensor("ident", [P, P], f32, kind="ExternalInput").ap()
    ind1_d = nc.dram_tensor("ind1", [5, CHI, P], f32, kind="ExternalInput").ap()
    ind2_d = nc.dram_tensor("ind2", [5, P, 32], f32, kind="ExternalInput").ap()
    y_d = nc.dram_tensor("y", [S, D], f32, kind="ExternalOutput").ap()

    pp = dict(nt=nt_bufs, wantf=2, wblk=2, scratch=2, osb=2,
              pst=5, pss=1, pso=2)
    if pools:
        pp.update(pools)

    with tile.TileContext(nc) as tc:
        with (
            tc.tile_pool(name="nt", bufs=pp["nt"]) as nt_pool,
            tc.tile_pool(name="wantf", bufs=pp["wantf"]) as wantf_pool,
            tc.tile_pool(name="wblk", bufs=pp["wblk"]) as wblk_pool,
            tc.tile_pool(name="scratch", bufs=pp["scratch"]) as s_pool,
            tc.tile_pool(name="small", bufs=6) as sm_pool,
            tc.tile_pool(name="rhs", bufs=3) as rhs_pool,
            tc.tile_pool(name="osb", bufs=pp["osb"]) as osb_pool,
            tc.tile_pool(name="singles", bufs=1) as ones_pool,
            tc.tile_pool(name="pst", bufs=pp["pst"], space="PSUM") as pst_pool,
            tc.tile_pool(name="pss", bufs=pp["pss"], space="PSUM") as pss_pool,
            tc.tile_pool(name="pso", bufs=pp["pso"], space="PSUM") as pso_pool,
        ):
            inv_chi_col = ones_pool.tile([P, 1], f32)
            nc.vector.memset(inv_chi_col, 1.0 / CHI)
            ident = ones_pool.tile([P, P], f32)
            nc.sync.dma_start(out=ident, in_=ident_d)
            ind1 = ones_pool.tile([CHI, 5, P], f32)
            nc.sync.dma_start(out=ind1, in_=ind1_d.rearrange("s c p -> c s p"))
            ind2 = ones_pool.tile([P, 5, 32], f32)
            nc.sync.dma_start(out=ind2, in_=ind2_d.rearrange("s p g -> p s g"))

            wantfs = [None] * S
            wblks = [None] * S
            rhss = [None] * S
            partials_arr = [None] * S
            scr = [None] * S

            def emit_s1_frame(b, c):
                """partials[:, c] = sum_f wf[:, c, f] * wf[:, 19, f]."""
                wf = wantfs[b].rearrange("p (c e) -> p c e", c=CHI)
                partials = partials_arr[b]
                if c == CHI - 1:
                    if sq_dve:
                        nc.vector.scalar_tensor_tensor(
                            out=scr[b][4],
                            in0=wf[:, CHI - 1, :],
                            scalar=1.0,
                            in1=wf[:, CHI - 1, :],
                            op0=mybir.AluOpType.mult,
                            op1=mybir.AluOpType.mult,
                            accum_out=partials[:, CHI - 1 : CHI],
                        )
                    else:
                        nc.scalar.activation(
                            out=scr[b][4],
                            in_=wf[:, CHI - 1, :],
                            func=mybir.ActivationFunctionType.Square,
                            accum_out=partials[:, CHI - 1 : CHI],
                        )
                    return
                nc.vector.scalar_tensor_tensor(
                    out=scr[b][c % 4],
                    in0=wf[:, c, :],
                    scalar=1.0,
                    in1=wf[:, CHI - 1, :],
                    op0=mybir.AluOpType.mult,
                    op1=mybir.AluOpType.mult,
                    accum_out=partials[:, c : c + 1],
                )

            def emit_stage1_dots(b):
                emit_s1_frame(b, CHI - 1)
                for c in range(CHI - 1):
                    emit_s1_frame(b, c)

            def emit_load_transpose(b):
                u = x_d[b].rearrange("(p q) -> p q", p=P)
                wantf = wantf_pool.tile([P, CHI * F], f32)
                wblk = wblk_pool.tile([P, NB * P], bf16)
                wantfs[b] = wantf
                wblks[b] = wblk
                partials_arr[b] = sm_pool.tile([P, CHI], f32, name="partials")
                scr[b] = [s_pool.tile([P, F], f32, name=f"scr{i}") for i in range(5)]
                wantf_v = wantf.rearrange("p (pp j) -> p j pp", j=NB)
                wblk_v = wblk.rearrange(
                    "p (q s t h dd) -> p q s t h dd", q=16, s=5, t=8, h=4
                )
                halves = [None, None]
                for h in range(4):
                    if big_chunks:
                        if h % 2 == 0:
                            nth = nt_pool.tile([P, 2 * HB * P], f32, name="nth")
                            nc.sync.dma_start(
                                out=nth,
                                in_=u[:, h * HB * P : (h + 2) * HB * P],
                            )
                            halves[h // 2] = nth
                        nt = halves[h // 2][:, (h % 2) * HB * P : (h % 2 + 1) * HB * P]
                    else:
                        nt = nt_pool.tile([P, HB * P], f32)
                        nc.sync.dma_start(
                            out=nt, in_=u[:, h * HB * P : (h + 1) * HB * P]
                        )
                    for sres in range(5):
                        # psum tile sres holds blocks j = 5*(4h+dd) + sres
                        ps = pst_pool.tile([P, 4, P], f32)
                        for dd in range(4):
                            jl = 5 * dd + sres  # local block within chunk h
                            nc.tensor.transpose(
                                ps[:, dd, :], nt[:, jl * P : (jl + 1) * P], ident
                            )
                        jga = h * 5 + sres
                        # flat f32 copy (for stage 1): dst j-blocks stride 5
                        j0 = 20 * h + sres
                        dst = wantf_v[:, j0 : j0 + 16 : 5, :]
                        if jga % 5 < n_dve_f32:
                            nc.vector.tensor_copy(out=dst, in_=ps)
                        else:
                            nc.scalar.copy(out=dst, in_=ps)
                        # bf16 copy for stage 2 (t-major block layout):
                        # wbs[p, 640q+128s+16t+4h+dd] = ps[p, dd, 8q+t]
                        bdst = wblk_v[:, :, sres, :, h, :]
                        if jga % 5 < -(-n_pool_bf // 4):
                            # Pool recast: SBUF->SBUF from the flat copy
                            src_b = dst.rearrange("p dd (q t) -> p q t dd", t=8)
                            nc.gpsimd.tensor_copy(out=bdst, in_=src_b)
                        else:
                            src_b = ps.rearrange("p dd (q t) -> p q t dd", t=8)
                            nc.scalar.copy(out=bdst, in_=src_b)

            def emit_stage1_rest(b):
                partials = partials_arr[b]
                soft = pss_pool.tile([P, 48], f32)
                s_psum = soft[0:1, 0:CHI]
                a_psum = soft[0:CHI, 24:25]
                a_pat = soft[:, 32:40]
                nc.tensor.matmul(s_psum, inv_chi_col, partials, start=True, stop=True)
                scores = sm_pool.tile([1, CHI], f32)
                nc.vector.tensor_copy(out=scores, in_=s_psum)

                neg_mx = sm_pool.tile([1, 1], f32)
                nc.vector.tensor_reduce(
                    out=neg_mx,
                    in_=scores,
                    axis=mybir.AxisListType.X,
                    op=mybir.AluOpType.max,
                    negate=True,
                )
                exps = sm_pool.tile([1, CHI], f32)
                sumexp = sm_pool.tile([1, 1], f32)
                nc.scalar.activation(
                    out=exps,
                    in_=scores,
                    func=mybir.ActivationFunctionType.Exp,
                    bias=neg_mx[:, 0:1],
                    scale=1.0,
                    accum_out=sumexp,
                )
                rsum = sm_pool.tile([1, 1], f32)
                nc.vector.reciprocal(rsum, sumexp)
                alpha = sm_pool.tile([1, CHI], f32)
                nc.vector.tensor_scalar_mul(alpha, exps, rsum)

                nc.tensor.transpose(a_psum, alpha, ident[:1, :1])
                a_one = sm_pool.tile([CHI, 1], f32)
                nc.scalar.copy(out=a_one, in_=a_psum)

                # a_pat[:, s] = ind1_s.T @ alpha_col = alpha[(128s+p) % 20]
                for s in range(NW):
                    nc.tensor.matmul(
                        a_pat[:, s : s + 1], ind1[:, s, :], a_one,
                        start=True, stop=True,
                    )
                # rhs_s[p, g] = ind2_s[p, g] * a_pat[p, s]  (bf16 for stage 2)
                rhs = rhs_pool.tile([P, NW, NG], bf16)
                rhss[b] = rhs
                if rhs_one:
                    a_bc = a_pat[:, 0:5].rearrange("p (s o) -> p s o", o=1)
                    nc.vector.tensor_mul(
                        rhs, ind2, a_bc.broadcast_to([P, NW, NG])
                    )
                else:
                    for s in range(NW):
                        nc.vector.tensor_scalar_mul(
                            rhs[:, s, :], ind2[:, s, :], a_pat[:, s : s + 1]
                        )

            def emit_stage2(b):
                wblk = wblks[b]
                rhs = rhss[b]
                ob = pso_pool.tile([P, NCHUNK, NG], f32)
                for W in range(NCHUNK):
                    for s in range(NW):
                        # stationary: contiguous 128 cols; free pos i=16t+jd
                        # holds want col 5*(128W + i) + s
                        stat = wblk[:, 640 * W + 128 * s : 640 * W + 128 * s + 128]
                        nc.tensor.matmul(
                            ob[:, W, :],
                            stat,
                            rhs[:, s, :],
                            start=(s == 0),
                            stop=(s == NW - 1),
                        )
                out_sb = osb_pool.tile([P, NCHUNK, NG], f32)
                nc.scalar.copy(out=out_sb, in_=ob)
                # psum partition i holds y[4096W + 32i + g]
                nc.sync.dma_start(
                    out=y_d[b].rearrange("(W m g) -> m W g", W=NCHUNK, g=NG),
                    in_=out_sb,
                )

            def emit_all():
                for b in range(S + 1):
                    if 0 <= b - 1 < S:
                        emit_stage1_dots(b - 1)
                    if b < S:
                        emit_load_transpose(b)
                    if 0 <= b - 1 < S:
                        emit_stage1_rest(b - 1)
                        emit_stage2(b - 1)

            if repeat == 1:
                emit_all()
            elif repeat < 0:  # dynamic loop (barrier per iteration)
                with tc.For_i(0, -repeat, 1):
                    emit_all()
            else:  # fully unrolled
                for _rep in range(repeat):
                    emit_all()

    nc.compile()
    return nc


def _host_inputs(xs):
    """Global (all-core concatenated) input arrays keyed by dram tensor name."""
    feed = {"x": xs}
    feed["ident"] = np.tile(np.eye(P, dtype=np.float32), (N_CORES, 1))
    s_idx = np.arange(5)[:, None]
    p_idx = np.arange(P)[None, :]
    cmap = (128 * s_idx + p_idx) % CHI  # [5, P]
    gmap = (128 * s_idx + p_idx) // CHI  # [5, P]
    ind1 = (np.arange(CHI)[None, :, None] == cmap[:, None, :]).astype(np.float32)
    ind2 = (np.arange(32)[None, None, :] == gmap[:, :, None]).astype(np.float32)
    feed["ind1"] = np.tile(ind1, (N_CORES, 1, 1))
    feed["ind2"] = np.tile(ind2, (N_CORES, 1, 1))
    return feed


def _get_nc():
    if "nc" not in _CACHE:
        _CACHE["nc"] = _build_nc_y()
    return _CACHE["nc"]


def _get_runner():
    if "runner" not in _CACHE:
        run, sharded, mesh, body = _make_runner(_get_nc())
        _CACHE["sharded"] = sharded
        _CACHE["mesh"] = mesh
        _CACHE["body"] = body
        _CACHE["runner"] = run
    return _CACHE["runner"]


def _make_runner(nc):
    """Compile once and return f(x_global[64, CHI*D]) -> y_global[64, D]."""
    import jax
    from jax.sharding import Mesh, PartitionSpec
    from jax.experimental.shard_map import shard_map
    from concourse import bass2jax, mybir

    bass2jax.install_neuronx_cc_hook()

    partition_name = (
        nc.partition_id_tensor.name if nc.partition_id_tensor else None
    )
    in_names = []
    out_names = []
    out_avals = []
    zero_outs = []
    for alloc in nc.m.functions[0].allocations:
        if not isinstance(alloc, mybir.MemoryLocationSet):
            continue
        name = alloc.memorylocations[0].name
        if alloc.kind == "ExternalInput":
            if name != partition_name:
                in_names.append(name)
        elif alloc.kind == "ExternalOutput":
            shape = tuple(alloc.tensor_shape)
            dtype = mybir.dt.np(alloc.dtype)
            out_avals.append(jax.core.ShapedArray(shape, dtype))
            out_names.append(name)
            zero_outs.append(np.zeros(shape, dtype))
    n_params = len(in_names)
    n_outs = len(out_avals)
    in_names.extend(out_names)
    donate = tuple(range(n_params, n_params + n_outs))

    def _body(*args):
        operands = list(args)
        if partition_name is not None:
            operands.append(bass2jax.partition_id_tensor())
            in_full = tuple(in_names) + (partition_name,)
        else:
            in_full = tuple(in_names)
        outs = bass2jax._bass_exec_p.bind(
            *operands,
            out_avals=tuple(out_avals),
            in_names=in_full,
            out_names=tuple(out_names),
            lowering_input_output_aliases=(),
            sim_require_finite=True,
            sim_require_nnan=True,
            nc=nc,
        )
        return tuple(outs)

    devices = jax.devices()[:N_CORES]
    mesh = Mesh(np.asarray(devices), ("core",))
    in_specs = (PartitionSpec("core"),) * (n_params + n_outs)
    out_specs = (PartitionSpec("core"),) * len(out_names)
    sharded = jax.jit(
        shard_map(
            _body, mesh=mesh, in_specs=in_specs, out_specs=out_specs, check_rep=False
        ),
        donate_argnums=donate,
        keep_unused=True,
    )

    param_names = in_names[:n_params]
    _CACHE["param_names"] = param_names

    def run(xs):
        feed = _host_inputs(xs)
        args = [feed[n] for n in param_names]
        concat_zeros = [
            np.zeros((N_CORES * z.shape[0], *z.shape[1:]), z.dtype) for z in zero_outs
        ]
        return sharded(*args, *concat_zeros)[0]

    return run, sharded, mesh, _body


def kernel(**inputs):
    x = np.ascontiguousarray(np.asarray(inputs["x"], dtype=np.float32))
    assert x.shape == (B, CHI, 64, 32, 32), x.shape
    xs = x.reshape(B, CHI * D)
    run = _get_runner()
    last_err = None
    for _attempt in range(3):
        try:
            out = np.asarray(run(xs))
            break
        except Exception as e:  # transient NRT device errors: retry
            last_err = e
    else:
        raise last_err
    return out.reshape(B, 64, 32, 32)


# revision 7
# speedup vs baseline: 1.0246x; 1.0079x over previous
"""ConvLSTM attention pooling kernel for 8 Trainium2 NeuronCores.

Reference computation (per sample b):
    frames = x[b].reshape(chi, D)            # D = C*H*W = 65536, chi = 20
    scores = frames @ frames[-1] / chi       # [chi]
    alpha  = softmax(scores)                 # [chi]
    y      = x[b].reshape(D, chi) @ alpha    # [D]  (interleaved reshape)

Sharding: pure data-parallel over batch B=64 -> 8 samples per core.

Architecture (v5, HW-measured on the axon-tunneled TRN2 cores):
  - one contiguous fp32 load of x[b] as nt[128, 10240] (DMA roofline
    ~124 us/core for the 42 MB shard)
  - 80 fp32 PE transposes/sample -> PSUM (fp32 transposes are fast on HW;
    bf16 transposes and fp32 matmuls are NOT - fp32 stage-2 matmuls
    measured 263 us/core standalone vs 9 us for bf16)
  - dual copies from each PSUM tile:
      * flat-f32 "want" layout (strided dst, fp32 full rate) for stage 1
      * t-major block-contiguous bf16 "wbs" layout for stage 2 stationaries
        (16-bit writes are ~4x slower when the dst is strided - the wbs
        layout keeps them in contiguous runs AND makes every stage-2
        stationary a contiguous 128-col slice)
      engine split: ACT most copies, DVE a share of flat, Pool (gpsimd)
      recasts a share of the bf16 groups SBUF->SBUF
  - stage 1: 19 DVE STT dot products + 1 ACT square (frame 19), column
    reduce on PE, softmax, alpha-scatter tiles (rhs, bf16)
  - stage 2: 80 bf16 matmuls (stat = wbs slice [128,128] contiguous,
    mov = rhs [128,32]), PSUM -> SBUF -> store
  HW repeat-R marginal: ~250 us/core (prior fp32 version: ~445 us/core).
"""

import numpy as np

B = 64
CHI = 20
D = 64 * 32 * 32  # 65536
N_CORES = 8
S = B // N_CORES  # samples per core
P = 128
F = D // P  # 512

NW = 5
NG = 32
NB = 80
NCHUNK = 16
HB = NB // 4  # 128-col blocks per load chunk

_CACHE = {}


def _build_nc_y(repeat=1, n_pool_bf=8, n_dve_f32=2, nt_bufs=4, pools=None,
                big_chunks=False, rhs_one=True, sq_dve=False, s2_lag=1):
    import concourse.bacc as bacc
    import concourse.tile as tile
    from concourse import mybir

    f32 = mybir.dt.float32
    bf16 = mybir.dt.bfloat16
    nc = bacc.Bacc("TRN2", target_bir_lowering=False, debug=False)
    x_d = nc.dram_tensor("x", [S, CHI * D], f32, kind="ExternalInput").ap()
    ident_d = nc.dram_tensor("ident", [P, P], f32, kind="ExternalInput").ap()
    ind1_d = nc.dram_tensor("ind1", [5, CHI, P], f32, kind="ExternalInput").ap()
    ind2_d = nc.dram_tensor("ind2", [5, P, 32], f32, kind="ExternalInput").ap()
    y_d = nc.dram_tensor("y", [S, D], f32, kind="ExternalOutput").ap()

    pp = dict(nt=nt_bufs, wantf=2, wblk=2, scratch=2, osb=2,
              pst=5, pss=1, pso=2)
    if pools:
        pp.update(pools)

    with tile.TileContext(nc) as tc:
        with (
            tc.tile_pool(name="nt", bufs=pp["nt"]) as nt_pool,
            tc.tile_pool(name="wantf", bufs=pp["wantf"]) as wantf_pool,
            tc.tile_pool(name="wblk", bufs=pp["wblk"]) as wblk_pool,
            tc.tile_pool(name="scratch", bufs=pp["scratch"]) as s_pool,
            tc.tile_pool(name="small", bufs=6) as sm_pool,
            tc.tile_pool(name="rhs", bufs=3) as rhs_pool,
            tc.tile_pool(name="osb", bufs=pp["osb"]) as osb_pool,
            tc.tile_pool(name="singles", bufs=1) as ones_pool,
            tc.tile_pool(name="pst", bufs=pp["pst"], space="PSUM") as pst_pool,
            tc.tile_pool(name="pss", bufs=pp["pss"], space="PSUM") as pss_pool,
            tc.tile_pool(name="pso", bufs=pp["pso"], space="PSUM") as pso_pool,
        ):
            inv_chi_col = ones_pool.tile([P, 1], f32)
            nc.vector.memset(inv_chi_col, 1.0 / CHI)
            ident = ones_pool.tile([P, P], f32)
            nc.sync.dma_start(out=ident, in_=ident_d)
            ind1 = ones_pool.tile([CHI, 5, P], f32)
            nc.sync.dma_start(out=ind1, in_=ind1_d.rearrange("s c p -> c s p"))
            ind2 = ones_pool.tile([P, 5, 32], f32)
            nc.sync.dma_start(out=ind2, in_=ind2_d.rearrange("s p g -> p s g"))

            wantfs = [None] * S
            wblks = [None] * S
            rhss = [None] * S
            partials_arr = [None] * S
            scr = [None] * S

            def emit_s1_frame(b, c):
                """partials[:, c] = sum_f wf[:, c, f] * wf[:, 19, f]."""
                wf = wantfs[b].rearrange("p (c e) -> p c e", c=CHI)
                partials = partials_arr[b]
                if c == CHI - 1:
                    if sq_dve:
                        nc.vector.scalar_tensor_tensor(
                            out=scr[b][4],
                            in0=wf[:, CHI - 1, :],
                            scalar=1.0,
                            in1=wf[:, CHI - 1, :],
                            op0=mybir.AluOpType.mult,
                            op1=mybir.AluOpType.mult,
                            accum_out=partials[:, CHI - 1 : CHI],
                        )
                    else:
                        nc.scalar.activation(
                            out=scr[b][4],
                            in_=wf[:, CHI - 1, :],
                            func=mybir.ActivationFunctionType.Square,
                            accum_out=partials[:, CHI - 1 : CHI],
                        )
                    return
                nc.vector.scalar_tensor_tensor(
                    out=scr[b][c % 4],
                    in0=wf[:, c, :],
                    scalar=1.0,
                    in1=wf[:, CHI - 1, :],
                    op0=mybir.AluOpType.mult,
                    op1=mybir.AluOpType.mult,
                    accum_out=partials[:, c : c + 1],
                )

            def emit_stage1_dots(b):
                emit_s1_frame(b, CHI - 1)
                for c in range(CHI - 1):
                    emit_s1_frame(b, c)

            def emit_load_transpose(b):
                u = x_d[b].rearrange("(p q) -> p q", p=P)
                wantf = wantf_pool.tile([P, CHI * F], f32)
                wblk = wblk_pool.tile([P, NB * P], bf16)
                wantfs[b] = wantf
                wblks[b] = wblk
                partials_arr[b] = sm_pool.tile([P, CHI], f32, name="partials")
                scr[b] = [s_pool.tile([P, F], f32, name=f"scr{i}") for i in range(5)]
                wantf_v = wantf.rearrange("p (pp j) -> p j pp", j=NB)
                wblk_v = wblk.rearrange(
                    "p (q s t h dd) -> p q s t h dd", q=16, s=5, t=8, h=4
                )
                halves = [None, None]
                for h in range(4):
                    if big_chunks:
                        if h % 2 == 0:
                            nth = nt_pool.tile([P, 2 * HB * P], f32, name="nth")
                            nc.sync.dma_start(
                                out=nth,
                                in_=u[:, h * HB * P : (h + 2) * HB * P],
                            )
                            halves[h // 2] = nth
                        nt = halves[h // 2][:, (h % 2) * HB * P : (h % 2 + 1) * HB * P]
                    else:
                        nt = nt_pool.tile([P, HB * P], f32)
                        nc.sync.dma_start(
                            out=nt, in_=u[:, h * HB * P : (h + 1) * HB * P]
                        )
                    for sres in range(5):
                        # psum tile sres holds blocks j = 5*(4h+dd) + sres
                        ps = pst_pool.tile([P, 4, P], f32)
                        for dd in range(4):
                            jl = 5 * dd + sres  # local block within chunk h
                            nc.tensor.transpose(
                                ps[:, dd, :], nt[:, jl * P : (jl + 1) * P], ident
                            )
                        jga = h * 5 + sres
                        # flat f32 copy (for stage 1): dst j-blocks stride 5
                        j0 = 20 * h + sres
                        dst = wantf_v[:, j0 : j0 + 16 : 5, :]
                        if jga % 5 < n_dve_f32:
                            nc.vector.tensor_copy(out=dst, in_=ps)
                        else:
                            nc.scalar.copy(out=dst, in_=ps)
                        # bf16 copy for stage 2 (t-major block layout):
                        # wbs[p, 640q+128s+16t+4h+dd] = ps[p, dd, 8q+t]
                        bdst = wblk_v[:, :, sres, :, h, :]
                        if jga % 5 < -(-n_pool_bf // 4):
                            # Pool recast: SBUF->SBUF from the flat copy
                            src_b = dst.rearrange("p dd (q t) -> p q t dd", t=8)
                            nc.gpsimd.tensor_copy(out=bdst, in_=src_b)
                        else:
                            src_b = ps.rearrange("p dd (q t) -> p q t dd", t=8)
                            nc.scalar.copy(out=bdst, in_=src_b)

            def emit_stage1_rest(b):
                partials = partials_arr[b]
                soft = pss_pool.tile([P, 48], f32)
                s_psum = soft[0:1, 0:CHI]
                a_psum = soft[0:CHI, 24:25]
                a_pat = soft[:, 32:40]
                nc.tensor.matmul(s_psum, inv_chi_col, partials, start=True, stop=True)
                scores = sm_pool.tile([1, CHI], f32)
                nc.vector.tensor_copy(out=scores, in_=s_psum)

                neg_mx = sm_pool.tile([1, 1], f32)
                nc.vector.tensor_reduce(
                    out=neg_mx,
                    in_=scores,
                    axis=mybir.AxisListType.X,
                    op=mybir.AluOpType.max,
                    negate=True,
                )
                exps = sm_pool.tile([1, CHI], f32)
                sumexp = sm_pool.tile([1, 1], f32)
                nc.scalar.activation(
                    out=exps,
                    in_=scores,
                    func=mybir.ActivationFunctionType.Exp,
                    bias=neg_mx[:, 0:1],
                    scale=1.0,
                    accum_out=sumexp,
                )
                rsum = sm_pool.tile([1, 1], f32)
                nc.vector.reciprocal(rsum, sumexp)
                alpha = sm_pool.tile([1, CHI], f32)
                nc.vector.tensor_scalar_mul(alpha, exps, rsum)

                nc.tensor.transpose(a_psum, alpha, ident[:1, :1])
                a_one = sm_pool.tile([CHI, 1], f32)
                nc.scalar.copy(out=a_one, in_=a_psum)

                # a_pat[:, s] = ind1_s.T @ alpha_col = alpha[(128s+p) % 20]
                for s in range(NW):
                    nc.tensor.matmul(
                        a_pat[:, s : s + 1], ind1[:, s, :], a_one,
                        start=True, stop=True,
                    )
                # rhs_s[p, g] = ind2_s[p, g] * a_pat[p, s]  (bf16 for stage 2)
                rhs = rhs_pool.tile([P, NW, NG], bf16)
                rhss[b] = rhs
                if rhs_one:
                    a_bc = a_pat[:, 0:5].rearrange("p (s o) -> p s o", o=1)
                    nc.vector.tensor_mul(
                        rhs, ind2, a_bc.broadcast_to([P, NW, NG])
                    )
                else:
                    for s in range(NW):
                        nc.vector.tensor_scalar_mul(
                            rhs[:, s, :], ind2[:, s, :], a_pat[:, s : s + 1]
                        )

            def emit_stage2(b):
                wblk = wblks[b]
                rhs = rhss[b]
                ob = pso_pool.tile([P, NCHUNK, NG], f32)
                for W in range(NCHUNK):
                    for s in range(NW):
                        # stationary: contiguous 128 cols; free pos i=16t+jd
                        # holds want col 5*(128W + i) + s
                        stat = wblk[:, 640 * W + 128 * s : 640 * W + 128 * s + 128]
                        nc.tensor.matmul(
                            ob[:, W, :],
                            stat,
                            rhs[:, s, :],
                            start=(s == 0),
                            stop=(s == NW - 1),
                        )
                out_sb = osb_pool.tile([P, NCHUNK, NG], f32)
                nc.scalar.copy(out=out_sb, in_=ob)
                # psum partition i holds y[4096W + 32i + g]
                nc.sync.dma_start(
                    out=y_d[b].rearrange("(W m g) -> m W g", W=NCHUNK, g=NG),
                    in_=out_sb,
                )

            def emit_all():
                for b in range(S + s2_lag):
                    if 0 <= b - 1 < S:
                        emit_stage1_dots(b - 1)
                    if b < S:
                        emit_load_transpose(b)
                    if 0 <= b - 1 < S:
                        emit_stage1_rest(b - 1)
                    if 0 <= b - s2_lag < S:
                        emit_stage2(b - s2_lag)

            if repeat == 1:
                emit_all()
            elif repeat < 0:  # dynamic loop (barrier per iteration)
                with tc.For_i(0, -repeat, 1):
                    emit_all()
            else:  # fully unrolled
                for _rep in range(repeat):
                    emit_all()

    nc.compile()
    return nc


def _host_inputs(xs):
    """Global (all-core concatenated) input arrays keyed by dram tensor name."""
    feed = {"x": xs}
    feed["ident"] = np.tile(np.eye(P, dtype=np.float32), (N_CORES, 1))
    s_idx = np.arange(5)[:, None]
    p_idx = np.arange(P)[None, :]
    cmap = (128 * s_idx + p_idx) % CHI  # [5, P]
    gmap = (128 * s_idx + p_idx) // CHI  # [5, P]
    ind1 = (np.arange(CHI)[None, :, None] == cmap[:, None, :]).astype(np.float32)
    ind2 = (np.arange(32)[None, None, :] == gmap[:, :, None]).astype(np.float32)
    feed["ind1"] = np.tile(ind1, (N_CORES, 1, 1))
    feed["ind2"] = np.tile(ind2, (N_CORES, 1, 1))
    return feed


def _get_nc():
    if "nc" not in _CACHE:
        _CACHE["nc"] = _build_nc_y()
    return _CACHE["nc"]


def _get_runner():
    if "runner" not in _CACHE:
        run, sharded, mesh, body = _make_runner(_get_nc())
        _CACHE["sharded"] = sharded
        _CACHE["mesh"] = mesh
        _CACHE["body"] = body
        _CACHE["runner"] = run
    return _CACHE["runner"]


def _make_runner(nc):
    """Compile once and return f(x_global[64, CHI*D]) -> y_global[64, D]."""
    import jax
    from jax.sharding import Mesh, PartitionSpec
    from jax.experimental.shard_map import shard_map
    from concourse import bass2jax, mybir

    bass2jax.install_neuronx_cc_hook()

    partition_name = (
        nc.partition_id_tensor.name if nc.partition_id_tensor else None
    )
    in_names = []
    out_names = []
    out_avals = []
    zero_outs = []
    for alloc in nc.m.functions[0].allocations:
        if not isinstance(alloc, mybir.MemoryLocationSet):
            continue
        name = alloc.memorylocations[0].name
        if alloc.kind == "ExternalInput":
            if name != partition_name:
                in_names.append(name)
        elif alloc.kind == "ExternalOutput":
            shape = tuple(alloc.tensor_shape)
            dtype = mybir.dt.np(alloc.dtype)
            out_avals.append(jax.core.ShapedArray(shape, dtype))
            out_names.append(name)
            zero_outs.append(np.zeros(shape, dtype))
    n_params = len(in_names)
    n_outs = len(out_avals)
    in_names.extend(out_names)
    donate = tuple(range(n_params, n_params + n_outs))

    def _body(*args):
        operands = list(args)
        if partition_name is not None:
            operands.append(bass2jax.partition_id_tensor())
            in_full = tuple(in_names) + (partition_name,)
        else:
            in_full = tuple(in_names)
        outs = bass2jax._bass_exec_p.bind(
            *operands,
            out_avals=tuple(out_avals),
            in_names=in_full,
            out_names=tuple(out_names),
            lowering_input_output_aliases=(),
            sim_require_finite=True,
            sim_require_nnan=True,
            nc=nc,
        )
        return tuple(outs)

    devices = jax.devices()[:N_CORES]
    mesh = Mesh(np.asarray(devices), ("core",))
    in_specs = (PartitionSpec("core"),) * (n_params + n_outs)
    out_specs = (PartitionSpec("core"),) * len(out_names)
    sharded = jax.jit(
        shard_map(
            _body, mesh=mesh, in_specs=in_specs, out_specs=out_specs, check_rep=False
        ),
        donate_argnums=donate,
        keep_unused=True,
    )

    param_names = in_names[:n_params]
    _CACHE["param_names"] = param_names

    def run(xs):
        feed = _host_inputs(xs)
        args = [feed[n] for n in param_names]
        concat_zeros = [
            np.zeros((N_CORES * z.shape[0], *z.shape[1:]), z.dtype) for z in zero_outs
        ]
        return sharded(*args, *concat_zeros)[0]

    return run, sharded, mesh, _body


def kernel(**inputs):
    x = np.ascontiguousarray(np.asarray(inputs["x"], dtype=np.float32))
    assert x.shape == (B, CHI, 64, 32, 32), x.shape
    xs = x.reshape(B, CHI * D)
    run = _get_runner()
    last_err = None
    for _attempt in range(3):
        try:
            out = np.asarray(run(xs))
            break
        except Exception as e:  # transient NRT device errors: retry
            last_err = e
    else:
        raise last_err
    return out.reshape(B, 64, 32, 32)


# revision 8
# speedup vs baseline: 1.0648x; 1.0393x over previous
"""ConvLSTM attention pooling kernel for 8 Trainium2 NeuronCores.

Reference computation (per sample b):
    frames = x[b].reshape(chi, D)            # D = C*H*W = 65536, chi = 20
    scores = frames @ frames[-1] / chi       # [chi]
    alpha  = softmax(scores)                 # [chi]
    y      = x[b].reshape(D, chi) @ alpha    # [D]  (interleaved reshape)

Sharding: pure data-parallel over batch B=64 -> 8 samples per core.

Architecture (v5, HW-measured on the axon-tunneled TRN2 cores):
  - one contiguous fp32 load of x[b] as nt[128, 10240] (DMA roofline
    ~124 us/core for the 42 MB shard)
  - 80 fp32 PE transposes/sample -> PSUM (fp32 transposes are fast on HW;
    bf16 transposes and fp32 matmuls are NOT - fp32 stage-2 matmuls
    measured 263 us/core standalone vs 9 us for bf16)
  - dual copies from each PSUM tile:
      * flat-f32 "want" layout (strided dst, fp32 full rate) for stage 1
      * t-major block-contiguous bf16 "wbs" layout for stage 2 stationaries
        (16-bit writes are ~4x slower when the dst is strided - the wbs
        layout keeps them in contiguous runs AND makes every stage-2
        stationary a contiguous 128-col slice)
      engine split: ACT most copies, DVE a share of flat, Pool (gpsimd)
      recasts a share of the bf16 groups SBUF->SBUF
  - stage 1: 19 DVE STT dot products + 1 ACT square (frame 19), column
    reduce on PE, softmax, alpha-scatter tiles (rhs, bf16)
  - stage 2: 80 bf16 matmuls (stat = wbs slice [128,128] contiguous,
    mov = rhs [128,32]), PSUM -> SBUF -> store
  HW repeat-R marginal: ~250 us/core (prior fp32 version: ~445 us/core).
"""

import numpy as np

B = 64
CHI = 20
D = 64 * 32 * 32  # 65536
N_CORES = 8
S = B // N_CORES  # samples per core
P = 128
F = D // P  # 512

NW = 5
NG = 32
NB = 80
NCHUNK = 16
HB = NB // 4  # 128-col blocks per load chunk

_CACHE = {}


def _build_nc_y(repeat=1, n_pool_bf=8, n_dve_f32=2, nt_bufs=4, pools=None,
                big_chunks=False, rhs_one=True, sq_dve=False, s2_lag=2):
    import concourse.bacc as bacc
    import concourse.tile as tile
    from concourse import mybir

    f32 = mybir.dt.float32
    bf16 = mybir.dt.bfloat16
    nc = bacc.Bacc("TRN2", target_bir_lowering=False, debug=False)
    x_d = nc.dram_tensor("x", [S, CHI * D], f32, kind="ExternalInput").ap()
    ident_d = nc.dram_tensor("ident", [P, P], f32, kind="ExternalInput").ap()
    ind1_d = nc.dram_tensor("ind1", [5, CHI, P], f32, kind="ExternalInput").ap()
    ind2_d = nc.dram_tensor("ind2", [5, P, 32], f32, kind="ExternalInput").ap()
    y_d = nc.dram_tensor("y", [S, D], f32, kind="ExternalOutput").ap()

    pp = dict(nt=nt_bufs, wantf=2, wblk=3, scratch=1, osb=2,
              pst=6, pss=1, pso=1)
    if pools:
        pp.update(pools)

    with tile.TileContext(nc) as tc:
        with (
            tc.tile_pool(name="nt", bufs=pp["nt"]) as nt_pool,
            tc.tile_pool(name="wantf", bufs=pp["wantf"]) as wantf_pool,
            tc.tile_pool(name="wblk", bufs=pp["wblk"]) as wblk_pool,
            tc.tile_pool(name="scratch", bufs=pp["scratch"]) as s_pool,
            tc.tile_pool(name="small", bufs=6) as sm_pool,
            tc.tile_pool(name="rhs", bufs=3) as rhs_pool,
            tc.tile_pool(name="osb", bufs=pp["osb"]) as osb_pool,
            tc.tile_pool(name="singles", bufs=1) as ones_pool,
            tc.tile_pool(name="pst", bufs=pp["pst"], space="PSUM") as pst_pool,
            tc.tile_pool(name="pss", bufs=pp["pss"], space="PSUM") as pss_pool,
            tc.tile_pool(name="pso", bufs=pp["pso"], space="PSUM") as pso_pool,
        ):
            inv_chi_col = ones_pool.tile([P, 1], f32)
            nc.vector.memset(inv_chi_col, 1.0 / CHI)
            ident = ones_pool.tile([P, P], f32)
            nc.sync.dma_start(out=ident, in_=ident_d)
            ind1 = ones_pool.tile([CHI, 5, P], f32)
            nc.sync.dma_start(out=ind1, in_=ind1_d.rearrange("s c p -> c s p"))
            ind2 = ones_pool.tile([P, 5, 32], f32)
            nc.sync.dma_start(out=ind2, in_=ind2_d.rearrange("s p g -> p s g"))

            wantfs = [None] * S
            wblks = [None] * S
            rhss = [None] * S
            partials_arr = [None] * S
            scr = [None] * S

            def emit_s1_frame(b, c):
                """partials[:, c] = sum_f wf[:, c, f] * wf[:, 19, f]."""
                wf = wantfs[b].rearrange("p (c e) -> p c e", c=CHI)
                partials = partials_arr[b]
                if c == CHI - 1:
                    if sq_dve:
                        nc.vector.scalar_tensor_tensor(
                            out=scr[b][4],
                            in0=wf[:, CHI - 1, :],
                            scalar=1.0,
                            in1=wf[:, CHI - 1, :],
                            op0=mybir.AluOpType.mult,
                            op1=mybir.AluOpType.mult,
                            accum_out=partials[:, CHI - 1 : CHI],
                        )
                    else:
                        nc.scalar.activation(
                            out=scr[b][4],
                            in_=wf[:, CHI - 1, :],
                            func=mybir.ActivationFunctionType.Square,
                            accum_out=partials[:, CHI - 1 : CHI],
                        )
                    return
                nc.vector.scalar_tensor_tensor(
                    out=scr[b][c % 4],
                    in0=wf[:, c, :],
                    scalar=1.0,
                    in1=wf[:, CHI - 1, :],
                    op0=mybir.AluOpType.mult,
                    op1=mybir.AluOpType.mult,
                    accum_out=partials[:, c : c + 1],
                )

            def emit_stage1_dots(b):
                emit_s1_frame(b, CHI - 1)
                for c in range(CHI - 1):
                    emit_s1_frame(b, c)

            def emit_load_transpose(b):
                u = x_d[b].rearrange("(p q) -> p q", p=P)
                wantf = wantf_pool.tile([P, CHI * F], f32)
                wblk = wblk_pool.tile([P, NB * P], bf16)
                wantfs[b] = wantf
                wblks[b] = wblk
                partials_arr[b] = sm_pool.tile([P, CHI], f32, name="partials")
                scr[b] = [s_pool.tile([P, F], f32, name=f"scr{i}") for i in range(5)]
                wantf_v = wantf.rearrange("p (pp j) -> p j pp", j=NB)
                wblk_v = wblk.rearrange(
                    "p (q s t h dd) -> p q s t h dd", q=16, s=5, t=8, h=4
                )
                halves = [None, None]
                for h in range(4):
                    if big_chunks:
                        if h % 2 == 0:
                            nth = nt_pool.tile([P, 2 * HB * P], f32, name="nth")
                            nc.sync.dma_start(
                                out=nth,
                                in_=u[:, h * HB * P : (h + 2) * HB * P],
                            )
                            halves[h // 2] = nth
                        nt = halves[h // 2][:, (h % 2) * HB * P : (h % 2 + 1) * HB * P]
                    else:
                        nt = nt_pool.tile([P, HB * P], f32)
                        nc.sync.dma_start(
                            out=nt, in_=u[:, h * HB * P : (h + 1) * HB * P]
                        )
                    for sres in range(5):
                        # psum tile sres holds blocks j = 5*(4h+dd) + sres
                        ps = pst_pool.tile([P, 4, P], f32)
                        for dd in range(4):
                            jl = 5 * dd + sres  # local block within chunk h
                            nc.tensor.transpose(
                                ps[:, dd, :], nt[:, jl * P : (jl + 1) * P], ident
                            )
                        jga = h * 5 + sres
                        # flat f32 copy (for stage 1): dst j-blocks stride 5
                        j0 = 20 * h + sres
                        dst = wantf_v[:, j0 : j0 + 16 : 5, :]
                        if jga % 5 < n_dve_f32:
                            nc.vector.tensor_copy(out=dst, in_=ps)
                        else:
                            nc.scalar.copy(out=dst, in_=ps)
                        # bf16 copy for stage 2 (t-major block layout):
                        # wbs[p, 640q+128s+16t+4h+dd] = ps[p, dd, 8q+t]
                        bdst = wblk_v[:, :, sres, :, h, :]
                        if jga % 5 < -(-n_pool_bf // 4):
                            # Pool recast: SBUF->SBUF from the flat copy
                            src_b = dst.rearrange("p dd (q t) -> p q t dd", t=8)
                            nc.gpsimd.tensor_copy(out=bdst, in_=src_b)
                        else:
                            src_b = ps.rearrange("p dd (q t) -> p q t dd", t=8)
                            nc.scalar.copy(out=bdst, in_=src_b)

            def emit_stage1_rest(b):
                partials = partials_arr[b]
                soft = pss_pool.tile([P, 48], f32)
                s_psum = soft[0:1, 0:CHI]
                a_psum = soft[0:CHI, 24:25]
                a_pat = soft[:, 32:40]
                nc.tensor.matmul(s_psum, inv_chi_col, partials, start=True, stop=True)
                scores = sm_pool.tile([1, CHI], f32)
                nc.vector.tensor_copy(out=scores, in_=s_psum)

                neg_mx = sm_pool.tile([1, 1], f32)
                nc.vector.tensor_reduce(
                    out=neg_mx,
                    in_=scores,
                    axis=mybir.AxisListType.X,
                    op=mybir.AluOpType.max,
                    negate=True,
                )
                exps = sm_pool.tile([1, CHI], f32)
                sumexp = sm_pool.tile([1, 1], f32)
                nc.scalar.activation(
                    out=exps,
                    in_=scores,
                    func=mybir.ActivationFunctionType.Exp,
                    bias=neg_mx[:, 0:1],
                    scale=1.0,
                    accum_out=sumexp,
                )
                rsum = sm_pool.tile([1, 1], f32)
                nc.vector.reciprocal(rsum, sumexp)
                alpha = sm_pool.tile([1, CHI], f32)
                nc.vector.tensor_scalar_mul(alpha, exps, rsum)

                nc.tensor.transpose(a_psum, alpha, ident[:1, :1])
                a_one = sm_pool.tile([CHI, 1], f32)
                nc.scalar.copy(out=a_one, in_=a_psum)

                # a_pat[:, s] = ind1_s.T @ alpha_col = alpha[(128s+p) % 20]
                for s in range(NW):
                    nc.tensor.matmul(
                        a_pat[:, s : s + 1], ind1[:, s, :], a_one,
                        start=True, stop=True,
                    )
                # rhs_s[p, g] = ind2_s[p, g] * a_pat[p, s]  (bf16 for stage 2)
                rhs = rhs_pool.tile([P, NW, NG], bf16)
                rhss[b] = rhs
                if rhs_one:
                    a_bc = a_pat[:, 0:5].rearrange("p (s o) -> p s o", o=1)
                    nc.vector.tensor_mul(
                        rhs, ind2, a_bc.broadcast_to([P, NW, NG])
                    )
                else:
                    for s in range(NW):
                        nc.vector.tensor_scalar_mul(
                            rhs[:, s, :], ind2[:, s, :], a_pat[:, s : s + 1]
                        )

            def emit_stage2(b):
                wblk = wblks[b]
                rhs = rhss[b]
                ob = pso_pool.tile([P, NCHUNK, NG], f32)
                for W in range(NCHUNK):
                    for s in range(NW):
                        # stationary: contiguous 128 cols; free pos i=16t+jd
                        # holds want col 5*(128W + i) + s
                        stat = wblk[:, 640 * W + 128 * s : 640 * W + 128 * s + 128]
                        nc.tensor.matmul(
                            ob[:, W, :],
                            stat,
                            rhs[:, s, :],
                            start=(s == 0),
                            stop=(s == NW - 1),
                        )
                out_sb = osb_pool.tile([P, NCHUNK, NG], f32)
                nc.scalar.copy(out=out_sb, in_=ob)
                # psum partition i holds y[4096W + 32i + g]
                nc.sync.dma_start(
                    out=y_d[b].rearrange("(W m g) -> m W g", W=NCHUNK, g=NG),
                    in_=out_sb,
                )

            def emit_all():
                for b in range(S + s2_lag):
                    if 0 <= b - 1 < S:
                        emit_stage1_dots(b - 1)
                    if b < S:
                        emit_load_transpose(b)
                    if 0 <= b - 1 < S:
                        emit_stage1_rest(b - 1)
                    if 0 <= b - s2_lag < S:
                        emit_stage2(b - s2_lag)

            if repeat == 1:
                emit_all()
            elif repeat < 0:  # dynamic loop (barrier per iteration)
                with tc.For_i(0, -repeat, 1):
                    emit_all()
            else:  # fully unrolled
                for _rep in range(repeat):
                    emit_all()

    nc.compile()
    return nc


def _host_inputs(xs):
    """Global (all-core concatenated) input arrays keyed by dram tensor name."""
    feed = {"x": xs}
    feed["ident"] = np.tile(np.eye(P, dtype=np.float32), (N_CORES, 1))
    s_idx = np.arange(5)[:, None]
    p_idx = np.arange(P)[None, :]
    cmap = (128 * s_idx + p_idx) % CHI  # [5, P]
    gmap = (128 * s_idx + p_idx) // CHI  # [5, P]
    ind1 = (np.arange(CHI)[None, :, None] == cmap[:, None, :]).astype(np.float32)
    ind2 = (np.arange(32)[None, None, :] == gmap[:, :, None]).astype(np.float32)
    feed["ind1"] = np.tile(ind1, (N_CORES, 1, 1))
    feed["ind2"] = np.tile(ind2, (N_CORES, 1, 1))
    return feed


def _get_nc():
    if "nc" not in _CACHE:
        _CACHE["nc"] = _build_nc_y()
    return _CACHE["nc"]


def _get_runner():
    if "runner" not in _CACHE:
        run, sharded, mesh, body = _make_runner(_get_nc())
        _CACHE["sharded"] = sharded
        _CACHE["mesh"] = mesh
        _CACHE["body"] = body
        _CACHE["runner"] = run
    return _CACHE["runner"]


def _make_runner(nc):
    """Compile once and return f(x_global[64, CHI*D]) -> y_global[64, D]."""
    import jax
    from jax.sharding import Mesh, PartitionSpec
    from jax.experimental.shard_map import shard_map
    from concourse import bass2jax, mybir

    bass2jax.install_neuronx_cc_hook()

    partition_name = (
        nc.partition_id_tensor.name if nc.partition_id_tensor else None
    )
    in_names = []
    out_names = []
    out_avals = []
    zero_outs = []
    for alloc in nc.m.functions[0].allocations:
        if not isinstance(alloc, mybir.MemoryLocationSet):
            continue
        name = alloc.memorylocations[0].name
        if alloc.kind == "ExternalInput":
            if name != partition_name:
                in_names.append(name)
        elif alloc.kind == "ExternalOutput":
            shape = tuple(alloc.tensor_shape)
            dtype = mybir.dt.np(alloc.dtype)
            out_avals.append(jax.core.ShapedArray(shape, dtype))
            out_names.append(name)
            zero_outs.append(np.zeros(shape, dtype))
    n_params = len(in_names)
    n_outs = len(out_avals)
    in_names.extend(out_names)
    donate = tuple(range(n_params, n_params + n_outs))

    def _body(*args):
        operands = list(args)
        if partition_name is not None:
            operands.append(bass2jax.partition_id_tensor())
            in_full = tuple(in_names) + (partition_name,)
        else:
            in_full = tuple(in_names)
        outs = bass2jax._bass_exec_p.bind(
            *operands,
            out_avals=tuple(out_avals),
            in_names=in_full,
            out_names=tuple(out_names),
            lowering_input_output_aliases=(),
            sim_require_finite=True,
            sim_require_nnan=True,
            nc=nc,
        )
        return tuple(outs)

    devices = jax.devices()[:N_CORES]
    mesh = Mesh(np.asarray(devices), ("core",))
    in_specs = (PartitionSpec("core"),) * (n_params + n_outs)
    out_specs = (PartitionSpec("core"),) * len(out_names)
    sharded = jax.jit(
        shard_map(
            _body, mesh=mesh, in_specs=in_specs, out_specs=out_specs, check_rep=False
        ),
        donate_argnums=donate,
        keep_unused=True,
    )

    param_names = in_names[:n_params]
    _CACHE["param_names"] = param_names

    def run(xs):
        feed = _host_inputs(xs)
        args = [feed[n] for n in param_names]
        concat_zeros = [
            np.zeros((N_CORES * z.shape[0], *z.shape[1:]), z.dtype) for z in zero_outs
        ]
        return sharded(*args, *concat_zeros)[0]

    return run, sharded, mesh, _body


def kernel(**inputs):
    x = np.ascontiguousarray(np.asarray(inputs["x"], dtype=np.float32))
    assert x.shape == (B, CHI, 64, 32, 32), x.shape
    xs = x.reshape(B, CHI * D)
    run = _get_runner()
    last_err = None
    for _attempt in range(3):
        try:
            out = np.asarray(run(xs))
            break
        except Exception as e:  # transient NRT device errors: retry
            last_err = e
    else:
        raise last_err
    return out.reshape(B, 64, 32, 32)


# revision 9
# speedup vs baseline: 1.1081x; 1.0407x over previous
"""ConvLSTM attention pooling kernel for 8 Trainium2 NeuronCores.

Reference computation (per sample b):
    frames = x[b].reshape(chi, D)            # D = C*H*W = 65536, chi = 20
    scores = frames @ frames[-1] / chi       # [chi]
    alpha  = softmax(scores)                 # [chi]
    y      = x[b].reshape(D, chi) @ alpha    # [D]  (interleaved reshape)

Sharding: pure data-parallel over batch B=64 -> 8 samples per core.

Architecture (v5, HW-measured on the axon-tunneled TRN2 cores):
  - one contiguous fp32 load of x[b] as nt[128, 10240] (DMA roofline
    ~124 us/core for the 42 MB shard)
  - 80 fp32 PE transposes/sample -> PSUM (fp32 transposes are fast on HW;
    bf16 transposes and fp32 matmuls are NOT - fp32 stage-2 matmuls
    measured 263 us/core standalone vs 9 us for bf16)
  - dual copies from each PSUM tile:
      * flat-f32 "want" layout (strided dst, fp32 full rate) for stage 1
      * t-major block-contiguous bf16 "wbs" layout for stage 2 stationaries
        (16-bit writes are ~4x slower when the dst is strided - the wbs
        layout keeps them in contiguous runs AND makes every stage-2
        stationary a contiguous 128-col slice)
      engine split: ACT most copies, DVE a share of flat, Pool (gpsimd)
      recasts a share of the bf16 groups SBUF->SBUF
  - stage 1: 19 DVE STT dot products + 1 ACT square (frame 19), column
    reduce on PE, softmax, alpha-scatter tiles (rhs, bf16)
  - stage 2: 80 bf16 matmuls (stat = wbs slice [128,128] contiguous,
    mov = rhs [128,32]), PSUM -> SBUF -> store
  HW repeat-R marginal: ~250 us/core (prior fp32 version: ~445 us/core).
"""

import numpy as np

B = 64
CHI = 20
D = 64 * 32 * 32  # 65536
N_CORES = 8
S = B // N_CORES  # samples per core
P = 128
F = D // P  # 512

NW = 5
NG = 32
NB = 80
NCHUNK = 16
HB = NB // 4  # 128-col blocks per load chunk

_CACHE = {}


def _build_nc_y(repeat=1, n_pool_bf=4, n_dve_f32=2, nt_bufs=4, pools=None,
                big_chunks=False, rhs_one=True, sq_dve=False, s2_lag=2):
    import concourse.bacc as bacc
    import concourse.tile as tile
    from concourse import mybir

    f32 = mybir.dt.float32
    bf16 = mybir.dt.bfloat16
    nc = bacc.Bacc("TRN2", target_bir_lowering=False, debug=False)
    x_d = nc.dram_tensor("x", [S, CHI * D], f32, kind="ExternalInput").ap()
    ident_d = nc.dram_tensor("ident", [P, P], f32, kind="ExternalInput").ap()
    ind1_d = nc.dram_tensor("ind1", [5, CHI, P], f32, kind="ExternalInput").ap()
    ind2_d = nc.dram_tensor("ind2", [5, P, 32], f32, kind="ExternalInput").ap()
    y_d = nc.dram_tensor("y", [S, D], f32, kind="ExternalOutput").ap()

    pp = dict(nt=nt_bufs, wantf=2, wblk=3, scratch=1, osb=2,
              pst=6, pss=1, pso=1)
    if pools:
        pp.update(pools)

    with tile.TileContext(nc) as tc:
        with (
            tc.tile_pool(name="nt", bufs=pp["nt"]) as nt_pool,
            tc.tile_pool(name="wantf", bufs=pp["wantf"]) as wantf_pool,
            tc.tile_pool(name="wblk", bufs=pp["wblk"]) as wblk_pool,
            tc.tile_pool(name="scratch", bufs=pp["scratch"]) as s_pool,
            tc.tile_pool(name="small", bufs=6) as sm_pool,
            tc.tile_pool(name="rhs", bufs=3) as rhs_pool,
            tc.tile_pool(name="osb", bufs=pp["osb"]) as osb_pool,
            tc.tile_pool(name="singles", bufs=1) as ones_pool,
            tc.tile_pool(name="pst", bufs=pp["pst"], space="PSUM") as pst_pool,
            tc.tile_pool(name="pss", bufs=pp["pss"], space="PSUM") as pss_pool,
            tc.tile_pool(name="pso", bufs=pp["pso"], space="PSUM") as pso_pool,
        ):
            inv_chi_col = ones_pool.tile([P, 1], f32)
            nc.vector.memset(inv_chi_col, 1.0 / CHI)
            ident = ones_pool.tile([P, P], f32)
            nc.sync.dma_start(out=ident, in_=ident_d)
            ind1 = ones_pool.tile([CHI, 5, P], f32)
            nc.sync.dma_start(out=ind1, in_=ind1_d.rearrange("s c p -> c s p"))
            ind2 = ones_pool.tile([P, 5, 32], f32)
            nc.sync.dma_start(out=ind2, in_=ind2_d.rearrange("s p g -> p s g"))

            wantfs = [None] * S
            wblks = [None] * S
            rhss = [None] * S
            partials_arr = [None] * S
            scr = [None] * S

            def emit_s1_frame(b, c):
                """partials[:, c] = sum_f wf[:, c, f] * wf[:, 19, f]."""
                wf = wantfs[b].rearrange("p (c e) -> p c e", c=CHI)
                partials = partials_arr[b]
                if c == CHI - 1:
                    if sq_dve:
                        nc.vector.scalar_tensor_tensor(
                            out=scr[b][4],
                            in0=wf[:, CHI - 1, :],
                            scalar=1.0,
                            in1=wf[:, CHI - 1, :],
                            op0=mybir.AluOpType.mult,
                            op1=mybir.AluOpType.mult,
                            accum_out=partials[:, CHI - 1 : CHI],
                        )
                    else:
                        nc.scalar.activation(
                            out=scr[b][4],
                            in_=wf[:, CHI - 1, :],
                            func=mybir.ActivationFunctionType.Square,
                            accum_out=partials[:, CHI - 1 : CHI],
                        )
                    return
                nc.vector.scalar_tensor_tensor(
                    out=scr[b][c % 4],
                    in0=wf[:, c, :],
                    scalar=1.0,
                    in1=wf[:, CHI - 1, :],
                    op0=mybir.AluOpType.mult,
                    op1=mybir.AluOpType.mult,
                    accum_out=partials[:, c : c + 1],
                )

            def emit_stage1_dots(b):
                emit_s1_frame(b, CHI - 1)
                for c in range(CHI - 1):
                    emit_s1_frame(b, c)

            def emit_load_transpose(b):
                u = x_d[b].rearrange("(p q) -> p q", p=P)
                wantf = wantf_pool.tile([P, CHI * F], f32)
                wblk = wblk_pool.tile([P, NB * P], bf16)
                wantfs[b] = wantf
                wblks[b] = wblk
                partials_arr[b] = sm_pool.tile([P, CHI], f32, name="partials")
                scr[b] = [s_pool.tile([P, F], f32, name=f"scr{i}") for i in range(5)]
                wantf_v = wantf.rearrange("p (pp j) -> p j pp", j=NB)
                wblk_v = wblk.rearrange(
                    "p (q s t h dd) -> p q s t h dd", q=16, s=5, t=8, h=4
                )
                halves = [None, None]
                for h in range(4):
                    if big_chunks:
                        if h % 2 == 0:
                            nth = nt_pool.tile([P, 2 * HB * P], f32, name="nth")
                            nc.sync.dma_start(
                                out=nth,
                                in_=u[:, h * HB * P : (h + 2) * HB * P],
                            )
                            halves[h // 2] = nth
                        nt = halves[h // 2][:, (h % 2) * HB * P : (h % 2 + 1) * HB * P]
                    else:
                        nt = nt_pool.tile([P, HB * P], f32)
                        nc.sync.dma_start(
                            out=nt, in_=u[:, h * HB * P : (h + 1) * HB * P]
                        )
                    for sres in range(5):
                        # psum tile sres holds blocks j = 5*(4h+dd) + sres
                        ps = pst_pool.tile([P, 4, P], f32)
                        for dd in range(4):
                            jl = 5 * dd + sres  # local block within chunk h
                            nc.tensor.transpose(
                                ps[:, dd, :], nt[:, jl * P : (jl + 1) * P], ident
                            )
                        jga = h * 5 + sres
                        # flat f32 copy (for stage 1): dst j-blocks stride 5
                        j0 = 20 * h + sres
                        dst = wantf_v[:, j0 : j0 + 16 : 5, :]
                        if jga % 5 < n_dve_f32:
                            nc.vector.tensor_copy(out=dst, in_=ps)
                        else:
                            nc.scalar.copy(out=dst, in_=ps)
                        # bf16 copy for stage 2 (t-major block layout):
                        # wbs[p, 640q+128s+16t+4h+dd] = ps[p, dd, 8q+t]
                        bdst = wblk_v[:, :, sres, :, h, :]
                        if jga % 5 < -(-n_pool_bf // 4):
                            # Pool recast: SBUF->SBUF from the flat copy
                            src_b = dst.rearrange("p dd (q t) -> p q t dd", t=8)
                            nc.gpsimd.tensor_copy(out=bdst, in_=src_b)
                        else:
                            src_b = ps.rearrange("p dd (q t) -> p q t dd", t=8)
                            nc.scalar.copy(out=bdst, in_=src_b)

            def emit_stage1_rest(b):
                partials = partials_arr[b]
                soft = pss_pool.tile([P, 48], f32)
                s_psum = soft[0:1, 0:CHI]
                a_psum = soft[0:CHI, 24:25]
                a_pat = soft[:, 32:40]
                nc.tensor.matmul(s_psum, inv_chi_col, partials, start=True, stop=True)
                scores = sm_pool.tile([1, CHI], f32)
                nc.vector.tensor_copy(out=scores, in_=s_psum)

                neg_mx = sm_pool.tile([1, 1], f32)
                nc.vector.tensor_reduce(
                    out=neg_mx,
                    in_=scores,
                    axis=mybir.AxisListType.X,
                    op=mybir.AluOpType.max,
                    negate=True,
                )
                exps = sm_pool.tile([1, CHI], f32)
                sumexp = sm_pool.tile([1, 1], f32)
                nc.scalar.activation(
                    out=exps,
                    in_=scores,
                    func=mybir.ActivationFunctionType.Exp,
                    bias=neg_mx[:, 0:1],
                    scale=1.0,
                    accum_out=sumexp,
                )
                rsum = sm_pool.tile([1, 1], f32)
                nc.vector.reciprocal(rsum, sumexp)
                alpha = sm_pool.tile([1, CHI], f32)
                nc.vector.tensor_scalar_mul(alpha, exps, rsum)

                nc.tensor.transpose(a_psum, alpha, ident[:1, :1])
                a_one = sm_pool.tile([CHI, 1], f32)
                nc.scalar.copy(out=a_one, in_=a_psum)

                # a_pat[:, s] = ind1_s.T @ alpha_col = alpha[(128s+p) % 20]
                for s in range(NW):
                    nc.tensor.matmul(
                        a_pat[:, s : s + 1], ind1[:, s, :], a_one,
                        start=True, stop=True,
                    )
                # rhs_s[p, g] = ind2_s[p, g] * a_pat[p, s]  (bf16 for stage 2)
                rhs = rhs_pool.tile([P, NW, NG], bf16)
                rhss[b] = rhs
                if rhs_one:
                    a_bc = a_pat[:, 0:5].rearrange("p (s o) -> p s o", o=1)
                    nc.vector.tensor_mul(
                        rhs, ind2, a_bc.broadcast_to([P, NW, NG])
                    )
                else:
                    for s in range(NW):
                        nc.vector.tensor_scalar_mul(
                            rhs[:, s, :], ind2[:, s, :], a_pat[:, s : s + 1]
                        )

            def emit_stage2(b):
                wblk = wblks[b]
                rhs = rhss[b]
                ob = pso_pool.tile([P, NCHUNK, NG], f32)
                for W in range(NCHUNK):
                    for s in range(NW):
                        # stationary: contiguous 128 cols; free pos i=16t+jd
                        # holds want col 5*(128W + i) + s
                        stat = wblk[:, 640 * W + 128 * s : 640 * W + 128 * s + 128]
                        nc.tensor.matmul(
                            ob[:, W, :],
                            stat,
                            rhs[:, s, :],
                            start=(s == 0),
                            stop=(s == NW - 1),
                        )
                out_sb = osb_pool.tile([P, NCHUNK, NG], f32)
                nc.scalar.copy(out=out_sb, in_=ob)
                # psum partition i holds y[4096W + 32i + g]
                nc.sync.dma_start(
                    out=y_d[b].rearrange("(W m g) -> m W g", W=NCHUNK, g=NG),
                    in_=out_sb,
                )

            def emit_all():
                for b in range(S + s2_lag):
                    if 0 <= b - 1 < S:
                        emit_stage1_dots(b - 1)
                    if b < S:
                        emit_load_transpose(b)
                    if 0 <= b - 1 < S:
                        emit_stage1_rest(b - 1)
                    if 0 <= b - s2_lag < S:
                        emit_stage2(b - s2_lag)

            if repeat == 1:
                emit_all()
            elif repeat < 0:  # dynamic loop (barrier per iteration)
                with tc.For_i(0, -repeat, 1):
                    emit_all()
            else:  # fully unrolled
                for _rep in range(repeat):
                    emit_all()

    nc.compile()
    return nc


def _host_inputs(xs):
    """Global (all-core concatenated) input arrays keyed by dram tensor name."""
    feed = {"x": xs}
    feed["ident"] = np.tile(np.eye(P, dtype=np.float32), (N_CORES, 1))
    s_idx = np.arange(5)[:, None]
    p_idx = np.arange(P)[None, :]
    cmap = (128 * s_idx + p_idx) % CHI  # [5, P]
    gmap = (128 * s_idx + p_idx) // CHI  # [5, P]
    ind1 = (np.arange(CHI)[None, :, None] == cmap[:, None, :]).astype(np.float32)
    ind2 = (np.arange(32)[None, None, :] == gmap[:, :, None]).astype(np.float32)
    feed["ind1"] = np.tile(ind1, (N_CORES, 1, 1))
    feed["ind2"] = np.tile(ind2, (N_CORES, 1, 1))
    return feed


def _get_nc():
    if "nc" not in _CACHE:
        _CACHE["nc"] = _build_nc_y()
    return _CACHE["nc"]


def _get_runner():
    if "runner" not in _CACHE:
        run, sharded, mesh, body = _make_runner(_get_nc())
        _CACHE["sharded"] = sharded
        _CACHE["mesh"] = mesh
        _CACHE["body"] = body
        _CACHE["runner"] = run
    return _CACHE["runner"]


def _make_runner(nc):
    """Compile once and return f(x_global[64, CHI*D]) -> y_global[64, D]."""
    import jax
    from jax.sharding import Mesh, PartitionSpec
    from jax.experimental.shard_map import shard_map
    from concourse import bass2jax, mybir

    bass2jax.install_neuronx_cc_hook()

    partition_name = (
        nc.partition_id_tensor.name if nc.partition_id_tensor else None
    )
    in_names = []
    out_names = []
    out_avals = []
    zero_outs = []
    for alloc in nc.m.functions[0].allocations:
        if not isinstance(alloc, mybir.MemoryLocationSet):
            continue
        name = alloc.memorylocations[0].name
        if alloc.kind == "ExternalInput":
            if name != partition_name:
                in_names.append(name)
        elif alloc.kind == "ExternalOutput":
            shape = tuple(alloc.tensor_shape)
            dtype = mybir.dt.np(alloc.dtype)
            out_avals.append(jax.core.ShapedArray(shape, dtype))
            out_names.append(name)
            zero_outs.append(np.zeros(shape, dtype))
    n_params = len(in_names)
    n_outs = len(out_avals)
    in_names.extend(out_names)
    donate = tuple(range(n_params, n_params + n_outs))

    def _body(*args):
        operands = list(args)
        if partition_name is not None:
            operands.append(bass2jax.partition_id_tensor())
            in_full = tuple(in_names) + (partition_name,)
        else:
            in_full = tuple(in_names)
        outs = bass2jax._bass_exec_p.bind(
            *operands,
            out_avals=tuple(out_avals),
            in_names=in_full,
            out_names=tuple(out_names),
            lowering_input_output_aliases=(),
            sim_require_finite=True,
            sim_require_nnan=True,
            nc=nc,
        )
        return tuple(outs)

    devices = jax.devices()[:N_CORES]
    mesh = Mesh(np.asarray(devices), ("core",))
    in_specs = (PartitionSpec("core"),) * (n_params + n_outs)
    out_specs = (PartitionSpec("core"),) * len(out_names)
    sharded = jax.jit(
        shard_map(
            _body, mesh=mesh, in_specs=in_specs, out_specs=out_specs, check_rep=False
        ),
        donate_argnums=donate,
        keep_unused=True,
    )

    param_names = in_names[:n_params]
    _CACHE["param_names"] = param_names

    def run(xs):
        feed = _host_inputs(xs)
        args = [feed[n] for n in param_names]
        concat_zeros = [
            np.zeros((N_CORES * z.shape[0], *z.shape[1:]), z.dtype) for z in zero_outs
        ]
        return sharded(*args, *concat_zeros)[0]

    return run, sharded, mesh, _body


def kernel(**inputs):
    x = np.ascontiguousarray(np.asarray(inputs["x"], dtype=np.float32))
    assert x.shape == (B, CHI, 64, 32, 32), x.shape
    xs = x.reshape(B, CHI * D)
    run = _get_runner()
    last_err = None
    for _attempt in range(3):
        try:
            out = np.asarray(run(xs))
            break
        except Exception as e:  # transient NRT device errors: retry
            last_err = e
    else:
        raise last_err
    return out.reshape(B, 64, 32, 32)
